# revision 5
# baseline (speedup 1.0000x reference)
"""Trainium2 Bass kernel for nn_NNFFTLayer (radix-R butterfly mix layer).

Reference computation (per position p, last dim N=8192):
    scale = tile(weights, R)                  # weights: [1024], R=8 -> [8192]
    y     = (scale * x).reshape(..., 64, 8, 16)   # [k, i, c]
    out[..., k, j, c] = sum_i lin_weights[j, i] * y[..., k, i, c]

Each 128-element chunk k of the last dim undergoes an independent linear map
M_km (km = k % 8) that folds the scale and the 8x8 mix:
    M_km[j*16+c', i*16+c] = L[j,i] * weights[km*128 + i*16 + c] * (c' == c)

Device strategy (feature-sharded over 8 cores, 8 chunks each):
  - host casts x to bf16 and transposes to X^T [8192 feat, 8192 pos]:
    HBM-bandwidth bound, so bf16 halves the bytes (~0.3% rel err, gate
    2e-2); the transposed feature-major layout eliminates all on-chip
    transposes AND gives maximal 16 KiB contiguous DMA descriptor lines
    (sustained HBM rate was measured to degrade with small descriptors).
  - core c handles feature rows [c*1024, (c+1)*1024): 8 slabs of one
    128-row chunk x 8192 positions; chunk km = slab index for every core,
    so each slab is 16 matmuls vs one resident stationary M_km^T
    (rhs = X^T slab, 512 positions per matmul) -> f32 PSUM,
    DVE/ACT copies (alternating) downcast PSUM -> bf16 out slab
  - DMA 2 MiB slabs in/out; host reassembles Y^T, transposes, upcasts.
  ~16 MiB in + 16 MiB out per core.
"""

import sys

if "/opt/trn_rl_repo" not in sys.path:
    sys.path.insert(0, "/opt/trn_rl_repo")

import numpy as np
import ml_dtypes

BF16 = ml_dtypes.bfloat16

P = 128
N = 8192
R = 8
TWO_R = 16
N_CHUNKS = N // P        # 64 feature chunks
KM = 1024 // P           # 8 distinct per-chunk matrices
N_CORES = 8
POS_TOTAL = 4 * 2048     # 8192 positions (batch*seq)
ROWS_PER_CORE = N // N_CORES          # 1024 feature rows per core
SLABS = ROWS_PER_CORE // P            # 8 slabs (= chunks) per core
HB = 512                              # matmul free size (1 PSUM bank f32)
NH = POS_TOTAL // HB                  # 16 h-blocks per slab

_CACHE = {}


def _build_nc():
    import concourse.bacc as bacc
    import concourse.mybir as mybir
    import concourse.tile as tile

    nc = bacc.Bacc("TRN2", target_bir_lowering=False, debug=False)
    f32 = mybir.dt.float32
    bf16 = mybir.dt.bfloat16
    # xs/out hold this core's rows of X^T / Y^T: [feature row, position]
    xs = nc.dram_tensor("xs", (ROWS_PER_CORE, POS_TOTAL), bf16, kind="ExternalInput")
    mt = nc.dram_tensor("mt", (P, KM * P), bf16, kind="ExternalInput")
    out = nc.dram_tensor("out", (ROWS_PER_CORE, POS_TOTAL), bf16, kind="ExternalOutput")

    HP = POS_TOTAL // 2      # 4096: half the positions -> 1 MiB sub-slabs

    with tile.TileContext(nc) as tc:
        with (
            tc.tile_pool(name="singles", bufs=1) as singles,
            tc.tile_pool(name="xin", bufs=6) as xin,
            tc.tile_pool(name="outp", bufs=6) as outp,
            tc.tile_pool(name="mm_ps", bufs=8, space="PSUM") as mm_ps,
        ):
            mt_sb = singles.tile([P, KM * P], bf16)
            nc.sync.dma_start(mt_sb[:], mt[:, :])

            # 16 sub-slabs: (feature chunk s, position half v) — finer
            # pipeline quanta shrink the fill/drain tails; 8 KiB
            # descriptor lines still sustain near-peak HBM rate
            for u in range(2 * SLABS):
                s, v = u // 2, u % 2
                xsb = xin.tile([P, HP], bf16)
                nc.sync.dma_start(
                    xsb[:], xs[s * P:(s + 1) * P, v * HP:(v + 1) * HP]
                )
                osb = outp.tile([P, HP], bf16)
                for h in range(HP // HB):
                    mm = mm_ps.tile([P, HB], f32)
                    nc.tensor.matmul(
                        mm[:],
                        lhsT=mt_sb[:, s * P:(s + 1) * P],
                        rhs=xsb[:, h * HB:(h + 1) * HB],
                        start=True, stop=True,
                    )
                    ceng = nc.vector.tensor_copy if h % 2 == 0 else nc.scalar.copy
                    ceng(osb[:, h * HB:(h + 1) * HB], mm[:])
                # last two sub-slabs drain on the SP ring (idle by then)
                seng = nc.sync if u >= 2 * SLABS - 2 else nc.scalar
                seng.dma_start(
                    out[s * P:(s + 1) * P, v * HP:(v + 1) * HP], osb[:]
                )

    # Strip the framework's const-register memsets and the entry all-engine
    # barrier: the memsets' GpSimd library load (~6us Q7 boot) gates the
    # barrier and delays kernel start, and with them gone the barrier
    # protects nothing — register init is per-engine (engines are in-order)
    # and the tile context's own semaphores carry all cross-engine deps.
    entry = nc.main_func.blocks[0]
    entry.instructions = [
        i for i in entry.instructions
        if not isinstance(i, (mybir.InstMemset, mybir.InstDrain,
                              mybir.InstEventSemaphore))
    ]

    nc.compile()
    return nc


def _get_nc():
    if "nc" not in _CACHE:
        _CACHE["nc"] = _build_nc()
    return _CACHE["nc"]


def build_mt(weights, lin_weights):
    """[P, KM*P] table; column block km holds M_km^T (matmul lhsT layout)."""
    L = np.asarray(lin_weights, np.float32)
    w = np.asarray(weights, np.float32)
    a = np.arange(P)   # out index within chunk: a = j*16 + c'
    b = np.arange(P)   # in  index within chunk: b = i*16 + c
    mix = L[a[:, None] // TWO_R, b[None, :] // TWO_R] * (
        (a[:, None] % TWO_R) == (b[None, :] % TWO_R)
    ).astype(np.float32)
    mt = np.zeros((P, KM * P), np.float32)
    for km in range(KM):
        M = mix * w[km * P + b][None, :]       # [a, b]
        mt[:, km * P:(km + 1) * P] = M.T       # lhsT[b, a] = M[a, b]
    return np.ascontiguousarray(mt)


def prep_in_maps(x, weights, lin_weights):
    xflat = np.asarray(x, np.float32).reshape(POS_TOTAL, N).astype(BF16)
    xT = np.ascontiguousarray(xflat.T)         # [N feat, POS_TOTAL]
    mt_host = build_mt(weights, lin_weights).astype(BF16)
    return [
        {"xs": xT[c * ROWS_PER_CORE:(c + 1) * ROWS_PER_CORE],
         "mt": mt_host}
        for c in range(N_CORES)
    ]


def unpack_out(res, shape):
    yT = np.concatenate(
        [res.results[c]["out"] for c in range(N_CORES)], axis=0
    )                                          # [N feat, POS_TOTAL] bf16
    return yT.T.astype(np.float32).reshape(shape)


def kernel(x, weights, lin_weights):
    from concourse import bass_utils

    nc = _get_nc()
    in_maps = prep_in_maps(x, weights, lin_weights)
    res = bass_utils.run_bass_kernel_spmd(nc, in_maps, core_ids=list(range(N_CORES)))
    return unpack_out(res, np.asarray(x).shape)
